# revision 9
# baseline (speedup 1.0000x reference)
"""Trainium2 Bass kernel: BinaryGraphClassifier (2x SGConv(k=3) + MLP head).

Strategy (8 NeuronCores, SPMD):
  - Nodes partitioned contiguously across cores (12500/core, padded to 12544).
  - Per SpMM hop: AllGather full node table H (bf16) -> every core gathers its
    edges' source rows with dma_gather (edges pre-sorted by dst tile / src bank
    on host), segment-sums them into PSUM via one-hot matmuls on the PE, and
    rescales by the symmetric degree norm folded into per-partition ACT scales.
  - Boundaries (after hop 3 and 6): PE transpose + W matmul + bias + ReLU.
  - Mean pooling: mask-weighted column-sum matmul -> AllReduce -> tiny head.
"""

import os
import sys
import hashlib

import numpy as np
import ml_dtypes

for _p in ("/opt/trn_rl_repo", "/root/.axon_site/_ro/trn_rl_repo"):
    if os.path.isdir(_p) and _p not in sys.path:
        sys.path.insert(0, _p)

import concourse.bass as bass  # noqa: E402
import concourse.mybir as mybir  # noqa: E402
from concourse import bacc  # noqa: E402
from concourse import tile  # noqa: E402
from concourse import bass_utils  # noqa: E402

BF16 = mybir.dt.bfloat16
F32 = mybir.dt.float32
I16 = mybir.dt.int16
AF = mybir.ActivationFunctionType
ALU = mybir.AluOpType
NBF = ml_dtypes.bfloat16
D = 128


class Cfg:
    def __init__(self, N=100000, E=3200000, C=8, chunk=3, banks=4):
        self.N, self.E, self.C = N, E, C
        assert N % C == 0
        self.NPC = N // C                       # owned nodes per core
        self.T = (self.NPC + 127) // 128        # dst tiles per core
        self.NPAD = self.T * 128                # padded nodes per core
        self.NTOT = self.NPAD * C               # rows of the gathered table
        self.BANKS = banks
        assert self.NTOT % banks == 0
        self.BROWS = self.NTOT // banks         # rows per gather bank
        assert self.BROWS <= 32768              # int16 index limit
        self.CHUNK = chunk                      # dst tiles per gather chunk

    def key(self):
        return (self.N, self.E, self.C, self.CHUNK, self.BANKS)


# --------------------------------------------------------------------------
# Host-side graph preprocessing (partitioning / CSR-style edge layout)
# --------------------------------------------------------------------------

def _layout(cfg, src, dst):
    """Compute the shared (SPMD-identical) static layout + per-edge slots."""
    C, T, B, NPC, NPAD = cfg.C, cfg.T, cfg.BANKS, cfg.NPC, cfg.NPAD
    E = len(src)

    d_core = dst // NPC
    d_loc = dst - d_core * NPC
    d_tile = d_loc >> 7
    d_code = d_loc & 127
    s_core = src // NPC
    s_pad = s_core * NPAD + (src - s_core * NPC)
    s_bank = s_pad // cfg.BROWS
    s_rel = s_pad - s_bank * cfg.BROWS

    key = ((d_core * T + d_tile) * B + s_bank).astype(np.int64)
    counts = np.bincount(key, minlength=C * T * B).reshape(C, T, B)
    # Per-(tile,bank) section size, shared across cores: multiple of 128 so
    # every gather call starts on a 128-slot (one matmul group) boundary.
    S_tb = np.maximum(((counts.max(axis=0) + 127) // 128) * 128, 128)

    sec_blk0 = np.zeros((T, B), np.int64)
    sec_col0 = np.zeros((T, B), np.int64)
    chunks = []
    blk = col = 0
    for c0 in range(0, T, cfg.CHUNK):
        tl = list(range(c0, min(c0 + cfg.CHUNK, T)))
        cb0, cc0 = blk, col
        calls = []
        tile_blocks = {t: [] for t in tl}
        for b in range(B):
            call_b0, call_c0, nidx = blk - cb0, col - cc0, 0
            for t in tl:
                s = int(S_tb[t, b])
                sec_blk0[t, b] = blk
                sec_col0[t, b] = col
                tile_blocks[t].extend(range(blk - cb0, blk - cb0 + s // 128))
                blk += s // 128
                col += s // 16
                nidx += s
            # ucode SWDGE ring holds 1024 descriptors; one dma_gather must fit.
            for off in range(0, nidx, 1024):
                p = min(1024, nidx - off)
                calls.append((b, call_c0 + off // 16, p, call_b0 + off // 128,
                              p // 128))
        chunks.append(dict(col0=cc0, cols=col - cc0, blk0=cb0, nblk=blk - cb0,
                           calls=calls, tiles=tl, tile_blocks=tile_blocks))

    lay = dict(
        chunks=chunks, NGRP=blk, TOTCOL=col,
        CBmax=max(ci["nblk"] for ci in chunks),
        COLmax=max(ci["cols"] for ci in chunks),
    )

    # per-edge slot assignment
    order = np.argsort(key, kind="stable")
    ksort = key[order]
    gstarts = np.zeros(C * T * B + 1, np.int64)
    np.cumsum(counts.reshape(-1), out=gstarts[1:])
    rank = np.arange(E, dtype=np.int64) - gstarts[ksort]
    et, eb = d_tile[order], s_bank[order]
    edge = dict(
        core=d_core[order],
        gblk=sec_blk0[et, eb] + rank // 128,
        prow=rank % 128,
        gcol=sec_col0[et, eb] + rank // 16,
        irow=rank % 16,
        code=d_code[order],
        rel=s_rel[order],
    )
    return lay, edge


def _prepare(cfg, inputs):
    x = np.ascontiguousarray(np.asarray(inputs["x"], np.float32))
    src = np.asarray(inputs["src"]).astype(np.int64)
    dst = np.asarray(inputs["dst"]).astype(np.int64)
    N, C, T, NPC, NPAD = cfg.N, cfg.C, cfg.T, cfg.NPC, cfg.NPAD

    deg = np.bincount(dst, minlength=N).astype(np.float32)
    norm = (np.clip(deg, 1.0, None) ** -0.5).astype(np.float32)
    nm2v = (norm * norm).astype(np.float32)

    lay, e = _layout(cfg, src, dst)

    codes = np.full((C, 128, lay["NGRP"]), 255, np.float32)
    codes[e["core"], e["prow"], e["gblk"]] = e["code"]
    codes_bf = codes.astype(NBF)

    idx16 = np.zeros((C, 16, lay["TOTCOL"]), np.int16)
    idx16[e["core"], e["irow"], e["gcol"]] = e["rel"].astype(np.int16)
    idx16 = np.ascontiguousarray(np.tile(idx16, (1, 8, 1)))

    def per_node_tiles(vec, pad, dt):
        a = np.full((C, NPAD), pad, np.float32)
        a[:, :NPC] = vec.reshape(C, NPC)
        return np.ascontiguousarray(a.reshape(C, T, 128).transpose(0, 2, 1)).astype(dt)

    nm_t = per_node_tiles(norm, 1.0, np.float32)
    nm2_t = per_node_tiles(nm2v, 1.0, np.float32)
    mask_t = per_node_tiles(np.ones(N, np.float32), 0.0, NBF)

    xp = np.zeros((C, NPAD, D), np.float32)
    xp[:, :NPC] = x.reshape(C, NPC, D)

    iotab = np.ascontiguousarray(
        np.broadcast_to(np.arange(128, dtype=np.float32), (128, 128))).astype(NBF)
    identb = np.eye(128, dtype=np.float32).astype(NBF)

    com = dict(
        iotab=iotab, identb=identb,
        W1b=np.asarray(inputs["W1"], np.float32).astype(NBF),
        b1b=np.asarray(inputs["b1"], np.float32).reshape(1, D).astype(NBF),
        W2b=np.asarray(inputs["W2"], np.float32).astype(NBF),
        b2b=np.asarray(inputs["b2"], np.float32).reshape(1, D).astype(NBF),
        Wf1f=np.ascontiguousarray(np.asarray(inputs["Wf1"], np.float32) / float(N)),
        bf1f=np.asarray(inputs["bf1"], np.float32).reshape(1, D).copy(),
        Wf2f=np.asarray(inputs["Wf2"], np.float32).reshape(D, 1).copy(),
        bf2f=np.asarray(inputs["bf2"], np.float32).reshape(1, 1).copy(),
    )
    in_maps = []
    for c in range(C):
        m = dict(com)
        m.update(x=np.ascontiguousarray(xp[c]),
                 nm=np.ascontiguousarray(nm_t[c]),
                 nm2=np.ascontiguousarray(nm2_t[c]),
                 maskb=np.ascontiguousarray(mask_t[c]),
                 codes=np.ascontiguousarray(codes_bf[c]),
                 idx=idx16[c])
        in_maps.append(m)
    return in_maps, lay


# --------------------------------------------------------------------------
# Kernel builder
# --------------------------------------------------------------------------

def _bcast_mid(ap_2d, n):
    """[P, W] AP -> [P, n, W] AP with a stride-0 middle dim."""
    return bass.AP(ap_2d.tensor, ap_2d.offset,
                   [ap_2d.ap[0], [0, n], ap_2d.ap[-1]])


def _bcast_inner(ap_2d, n):
    """[P, W] AP -> [P, W, n] AP with a stride-0 inner dim."""
    return bass.AP(ap_2d.tensor, ap_2d.offset,
                   [ap_2d.ap[0], ap_2d.ap[-1], [0, n]])


def _build(cfg, lay):
    C, T = cfg.C, cfg.T
    NPAD, NTOT, BROWS = cfg.NPAD, cfg.NTOT, cfg.BROWS
    RG = [list(range(C))]
    CBmax, COLmax = lay["CBmax"], lay["COLmax"]

    nc = bacc.Bacc("TRN2", target_bir_lowering=False, debug=False,
                   enable_asserts=False, num_devices=C,
                   num_swdge_queues=4)

    x_d = nc.dram_tensor("x", [NPAD, D], F32, kind="ExternalInput")
    nm_d = nc.dram_tensor("nm", [128, T], F32, kind="ExternalInput")
    nm2_d = nc.dram_tensor("nm2", [128, T], F32, kind="ExternalInput")
    mask_d = nc.dram_tensor("maskb", [128, T], BF16, kind="ExternalInput")
    codes_d = nc.dram_tensor("codes", [128, lay["NGRP"]], BF16, kind="ExternalInput")
    idx_d = nc.dram_tensor("idx", [128, lay["TOTCOL"]], I16, kind="ExternalInput")
    iota_d = nc.dram_tensor("iotab", [128, 128], BF16, kind="ExternalInput")
    ident_d = nc.dram_tensor("identb", [128, 128], BF16, kind="ExternalInput")
    W1_d = nc.dram_tensor("W1b", [D, D], BF16, kind="ExternalInput")
    b1_d = nc.dram_tensor("b1b", [1, D], BF16, kind="ExternalInput")
    W2_d = nc.dram_tensor("W2b", [D, D], BF16, kind="ExternalInput")
    b2_d = nc.dram_tensor("b2b", [1, D], BF16, kind="ExternalInput")
    Wf1_d = nc.dram_tensor("Wf1f", [D, D], F32, kind="ExternalInput")
    bf1_d = nc.dram_tensor("bf1f", [1, D], F32, kind="ExternalInput")
    Wf2_d = nc.dram_tensor("Wf2f", [D, 1], F32, kind="ExternalInput")
    bf2_d = nc.dram_tensor("bf2f", [1, 1], F32, kind="ExternalInput")
    out_d = nc.dram_tensor("out", [1, 1], F32, kind="ExternalOutput")

    with tile.TileContext(nc) as tc:
        with (
            tc.tile_pool(name="const", bufs=1) as const,
            tc.tile_pool(name="dloc", bufs=2, space="DRAM") as dloc,
            tc.tile_pool(name="dsh", bufs=2, space="DRAM") as dsh,
            tc.tile_pool(name="xp", bufs=3) as xp,
            tc.tile_pool(name="evp", bufs=3) as evp,
            tc.tile_pool(name="gp", bufs=2) as gp,
            tc.tile_pool(name="sp", bufs=2) as sp,
            tc.tile_pool(name="ipool", bufs=2) as ipool,
            tc.tile_pool(name="ph", bufs=3, space="PSUM") as ph,
            tc.tile_pool(name="px", bufs=2, space="PSUM") as px,
            tc.tile_pool(name="pp", bufs=1, space="PSUM") as pp,
        ):
            # ---- resident constants
            nm_t = const.tile([128, T], F32)
            nc.sync.dma_start(out=nm_t[:], in_=nm_d[:, :])
            nm2_t = const.tile([128, T], F32)
            nc.sync.dma_start(out=nm2_t[:], in_=nm2_d[:, :])
            mask_t = const.tile([128, T], BF16)
            nc.sync.dma_start(out=mask_t[:], in_=mask_d[:, :])
            codes_t = const.tile([128, lay["NGRP"]], BF16)
            nc.sync.dma_start(out=codes_t[:], in_=codes_d[:, :])
            iota_t = const.tile([128, 128], BF16)
            nc.sync.dma_start(out=iota_t[:], in_=iota_d[:, :])
            ident_t = const.tile([128, 128], BF16)
            nc.sync.dma_start(out=ident_t[:], in_=ident_d[:, :])
            w1_t = const.tile([D, D], BF16)
            nc.sync.dma_start(out=w1_t[:], in_=W1_d[:, :])
            b1_t = const.tile([1, D], BF16)
            nc.sync.dma_start(out=b1_t[:1, :], in_=b1_d[:, :])
            w2_t = const.tile([D, D], BF16)
            nc.sync.dma_start(out=w2_t[:], in_=W2_d[:, :])
            b2_t = const.tile([1, D], BF16)
            nc.sync.dma_start(out=b2_t[:1, :], in_=b2_d[:, :])
            wf1_t = const.tile([D, D], F32)
            nc.sync.dma_start(out=wf1_t[:], in_=Wf1_d[:, :])
            bf1_t = const.tile([1, D], F32)
            nc.sync.dma_start(out=bf1_t[:1, :], in_=bf1_d[:, :])
            wf2_t = const.tile([D, 1], F32)
            nc.sync.dma_start(out=wf2_t[:], in_=Wf2_d[:, :])
            bf2_t = const.tile([1, 1], F32)
            nc.sync.dma_start(out=bf2_t[:1, :1], in_=bf2_d[:, :])
            ones_b = const.tile([1, D], BF16)
            nc.vector.memset(ones_b[:1, :], 1.0)
            one11 = const.tile([1, 1], F32)
            nc.vector.memset(one11[:1, :1], 1.0)

            pool_ps = pp.tile([128, 1], F32)

            # ---- prologue: g0 = norm * x
            hloc = dloc.tile([NPAD, D], BF16, name="hloc")
            for t in range(T):
                xt = xp.tile([128, D], F32, name="xt")
                nc.sync.dma_start(out=xt[:], in_=x_d[t * 128:(t + 1) * 128, :])
                ev = evp.tile([128, D], BF16, name="ev")
                nc.scalar.activation(ev[:], xt[:], AF.Copy, scale=nm_t[:, t:t + 1])
                nc.sync.dma_start(out=hloc[t * 128:(t + 1) * 128, :], in_=ev[:])
            hfull = dsh.tile([NTOT, D], BF16, addr_space="Shared", name="hfull")
            nc.gpsimd.collective_compute(
                "AllGather", ALU.bypass, replica_groups=RG,
                ins=[hloc.opt()], outs=[hfull.opt()])

            # ---- 6 propagation hops
            qn = 0
            for hop in range(1, 7):
                boundary = hop in (3, 6)
                wt, bt = (w1_t, b1_t) if hop == 3 else (w2_t, b2_t)
                hl_new = None
                if hop < 6:
                    hl_new = dloc.tile([NPAD, D], BF16, name="hloc")
                for info in lay["chunks"]:
                    idxt = ipool.tile([128, COLmax], I16, name="idxt")
                    nc.sync.dma_start(
                        out=idxt[:, :info["cols"]],
                        in_=idx_d[:, info["col0"]:info["col0"] + info["cols"]])
                    G = gp.tile([128, CBmax, D], BF16, name="G")
                    for (b, ccol0, nidx, cblk0, nblk) in info["calls"]:
                        nc.gpsimd.dma_gather(
                            out_ap=G[:, cblk0:cblk0 + nblk, :],
                            in_ap=hfull[b * BROWS:(b + 1) * BROWS, :],
                            idxs_ap=idxt[:, ccol0:ccol0 + nidx // 16],
                            num_idxs=nidx, num_idxs_reg=nidx, elem_size=D,
                            queue_num=qn % 4)
                        qn += 1
                    S = sp.tile([128, CBmax * D], BF16, name="S")
                    nb = info["nblk"]
                    s3 = S[:, :nb * D].rearrange("p (g d) -> p g d", d=D)
                    nc.vector.tensor_tensor(
                        s3, _bcast_mid(iota_t[:, :], nb),
                        _bcast_inner(codes_t[:, info["blk0"]:info["blk0"] + nb], D),
                        ALU.is_equal)
                    for t in info["tiles"]:
                        blks = info["tile_blocks"][t]
                        ps = ph.tile([128, D], F32, name="ps")
                        for j, bl in enumerate(blks):
                            nc.tensor.matmul(
                                ps[:], S[:, bl * D:(bl + 1) * D], G[:, bl, :],
                                start=(j == 0), stop=(j == len(blks) - 1))
                        if not boundary:
                            evh = evp.tile([128, D], BF16, name="evh")
                            nc.scalar.activation(evh[:], ps[:], AF.Copy,
                                                 scale=nm2_t[:, t:t + 1])
                            nc.sync.dma_start(
                                out=hl_new[t * 128:(t + 1) * 128, :], in_=evh[:])
                        else:
                            hv = evp.tile([128, D], BF16, name="hv")
                            nc.scalar.activation(hv[:], ps[:], AF.Copy,
                                                 scale=nm_t[:, t:t + 1])
                            tp = px.tile([128, D], BF16, name="tp")
                            nc.tensor.transpose(tp[:], hv[:], ident_t[:])
                            hT = evp.tile([128, D], BF16, name="hT")
                            nc.vector.tensor_copy(hT[:], tp[:])
                            wp = px.tile([128, D], F32, name="wp")
                            nc.tensor.matmul(wp[:], hT[:], wt[:],
                                             start=True, stop=False)
                            nc.tensor.matmul(wp[:], ones_b[:1, :], bt[:1, :],
                                             start=False, stop=True)
                            ev2 = evp.tile([128, D], BF16, name="ev2")
                            if hop == 3:
                                nc.scalar.activation(ev2[:], wp[:], AF.Relu,
                                                     scale=nm_t[:, t:t + 1])
                                nc.sync.dma_start(
                                    out=hl_new[t * 128:(t + 1) * 128, :], in_=ev2[:])
                            else:
                                nc.scalar.activation(ev2[:], wp[:], AF.Relu)
                                nc.tensor.matmul(
                                    pool_ps[:], ev2[:], mask_t[:, t:t + 1],
                                    start=(t == 0), stop=(t == T - 1),
                                    skip_group_check=True)
                if hop < 6:
                    hfull = dsh.tile([NTOT, D], BF16, addr_space="Shared",
                                     name="hfull")
                    nc.gpsimd.collective_compute(
                        "AllGather", ALU.bypass, replica_groups=RG,
                        ins=[hl_new.opt()], outs=[hfull.opt()])

            # ---- head: AllReduce pooled sums, 2-layer MLP, sigmoid
            hgs = evp.tile([128, 1], F32, name="hgs")
            nc.vector.tensor_copy(hgs[:], pool_ps[:])
            arin = dloc.tile([128, 1], F32, name="arin")
            nc.sync.dma_start(out=arin[:, :], in_=hgs[:])
            arout = dsh.tile([128, 1], F32, addr_space="Shared", name="arout")
            nc.gpsimd.collective_compute(
                "AllReduce", ALU.add, replica_groups=RG,
                ins=[arin.opt()], outs=[arout.opt()])
            hgT = evp.tile([128, 1], F32, name="hgT")
            nc.sync.dma_start(out=hgT[:, :], in_=arout[:, :])

            py1 = px.tile([1, D], F32, name="py1", tag="wp")
            nc.tensor.matmul(py1[:1, :], hgT[:], wf1_t[:],
                             start=True, stop=False, skip_group_check=True)
            nc.tensor.matmul(py1[:1, :], one11[:1, :1], bf1_t[:1, :],
                             start=False, stop=True, skip_group_check=True)
            y1 = evp.tile([1, D], F32, name="y1")
            nc.scalar.activation(y1[:1, :], py1[:1, :], AF.Relu)
            py1T = px.tile([128, 1], F32, name="py1T", tag="wp")
            nc.tensor.transpose(py1T[:, :1], y1[:1, :], one11[:1, :1])
            y1T = evp.tile([128, 1], F32, name="y1T")
            nc.vector.tensor_copy(y1T[:], py1T[:, :1])
            py2 = px.tile([1, 1], F32, name="py2", tag="wp")
            nc.tensor.matmul(py2[:1, :1], wf2_t[:], y1T[:],
                             start=True, stop=False, skip_group_check=True)
            nc.tensor.matmul(py2[:1, :1], one11[:1, :1], bf2_t[:1, :1],
                             start=False, stop=True, skip_group_check=True)
            r2 = evp.tile([1, 1], F32, name="r2")
            nc.scalar.activation(r2[:1, :1], py2[:1, :1], AF.Relu)
            osb = evp.tile([1, 1], F32, name="osb")
            nc.scalar.activation(osb[:1, :1], r2[:1, :1], AF.Sigmoid)
            nc.sync.dma_start(out=out_d[:, :], in_=osb[:1, :1])

    nc.compile()
    return nc


# --------------------------------------------------------------------------
# Runner
# --------------------------------------------------------------------------

_NC_CACHE = {}


def _ensure_ntff_hook():
    """bass_utils imports antenv.axon_hooks for trace=True under axon; the
    image's antenv lacks it. Provide it + register the ctypes NTFF hook."""
    import types
    try:
        from antenv import axon_hooks  # noqa: F401
        return
    except ImportError:
        pass
    import antenv
    mod = types.ModuleType("antenv.axon_hooks")
    _h = {"h": None}
    mod.set_axon_ntff_profile_hook = lambda h: _h.__setitem__("h", h)
    mod.get_axon_ntff_profile_hook = lambda: _h["h"]
    sys.modules["antenv.axon_hooks"] = mod
    antenv.axon_hooks = mod
    try:
        if "/root/.axon_site" not in sys.path:
            sys.path.insert(0, "/root/.axon_site")
        from trn_agent_boot.trn_boot import _ntff_profile_via_ctypes
        h = _ntff_profile_via_ctypes("/opt/axon/libaxon_pjrt.so")
        if h is not None:
            mod.set_axon_ntff_profile_hook(h)
    except Exception as e:  # pragma: no cover
        print("ntff hook setup failed:", e)


def _graph_key(cfg, src, dst):
    h = hashlib.sha1()
    h.update(np.asarray(src).tobytes())
    h.update(np.asarray(dst).tobytes())
    return (cfg.key(), h.hexdigest())


def run(inputs, cfg=None, trace=False, **kw):
    cfg = cfg or Cfg()
    if trace:
        _ensure_ntff_hook()
    in_maps, lay = _prepare(cfg, inputs)
    gk = _graph_key(cfg, inputs["src"], inputs["dst"])
    nc = _NC_CACHE.get(gk)
    if nc is None:
        nc = _build(cfg, lay)
        _NC_CACHE[gk] = nc
    res = bass_utils.run_bass_kernel_spmd(
        nc, in_maps, core_ids=list(range(cfg.C)), trace=trace, **kw)
    out = np.asarray(res.results[0]["out"], np.float32).reshape(1, 1)
    return out, res


def kernel(**inputs):
    out, _ = run(inputs)
    return out


# revision 11
# speedup vs baseline: 1.1402x; 1.1402x over previous
"""Trainium2 Bass kernel: BinaryGraphClassifier (2x SGConv(k=3) + MLP head).

Strategy (8 NeuronCores, SPMD):
  - Nodes partitioned contiguously across cores (12500/core, padded to 12544).
  - Per SpMM hop: AllGather full node table H (bf16) -> every core gathers its
    edges' source rows with dma_gather (edges pre-sorted by dst tile / src bank
    on host), segment-sums them into PSUM via one-hot matmuls on the PE, and
    rescales by the symmetric degree norm folded into per-partition ACT scales.
  - Boundaries (after hop 3 and 6): PE transpose + W matmul + bias + ReLU.
  - Mean pooling: mask-weighted column-sum matmul -> AllReduce -> tiny head.
"""

import os
import sys
import hashlib

import numpy as np
import ml_dtypes

for _p in ("/opt/trn_rl_repo", "/root/.axon_site/_ro/trn_rl_repo"):
    if os.path.isdir(_p) and _p not in sys.path:
        sys.path.insert(0, _p)

import concourse.bass as bass  # noqa: E402
import concourse.mybir as mybir  # noqa: E402
from concourse import bacc  # noqa: E402
from concourse import tile  # noqa: E402
from concourse import bass_utils  # noqa: E402

BF16 = mybir.dt.bfloat16
F32 = mybir.dt.float32
I16 = mybir.dt.int16
AF = mybir.ActivationFunctionType
ALU = mybir.AluOpType
NBF = ml_dtypes.bfloat16
D = 128


class Cfg:
    def __init__(self, N=100000, E=3200000, C=8, chunk=3, banks=4):
        self.N, self.E, self.C = N, E, C
        assert N % C == 0
        self.NPC = N // C                       # owned nodes per core
        self.T = (self.NPC + 127) // 128        # dst tiles per core
        self.NPAD = self.T * 128                # padded nodes per core
        self.NTOT = self.NPAD * C               # rows of the gathered table
        self.BANKS = banks
        assert self.NTOT % banks == 0
        self.BROWS = self.NTOT // banks         # rows per gather bank
        assert self.BROWS <= 32768              # int16 index limit
        self.CHUNK = chunk                      # dst tiles per gather chunk

    def key(self):
        return (self.N, self.E, self.C, self.CHUNK, self.BANKS)


# --------------------------------------------------------------------------
# Host-side graph preprocessing (partitioning / CSR-style edge layout)
# --------------------------------------------------------------------------

def _layout(cfg, src, dst):
    """Compute the shared (SPMD-identical) static layout + per-edge slots."""
    C, T, B, NPC, NPAD = cfg.C, cfg.T, cfg.BANKS, cfg.NPC, cfg.NPAD
    E = len(src)

    d_core = dst // NPC
    d_loc = dst - d_core * NPC
    d_tile = d_loc >> 7
    d_code = d_loc & 127
    s_core = src // NPC
    s_pad = s_core * NPAD + (src - s_core * NPC)
    s_bank = s_pad // cfg.BROWS
    s_rel = s_pad - s_bank * cfg.BROWS

    key = ((d_core * T + d_tile) * B + s_bank).astype(np.int64)
    counts = np.bincount(key, minlength=C * T * B).reshape(C, T, B)
    # Per-(tile,bank) section size, shared across cores: multiple of 128 so
    # every gather call starts on a 128-slot (one matmul group) boundary.
    S_tb = np.maximum(((counts.max(axis=0) + 127) // 128) * 128, 128)

    sec_blk0 = np.zeros((T, B), np.int64)
    sec_col0 = np.zeros((T, B), np.int64)
    chunks = []
    blk = col = 0
    for c0 in range(0, T, cfg.CHUNK):
        tl = list(range(c0, min(c0 + cfg.CHUNK, T)))
        cb0, cc0 = blk, col
        calls = []
        tile_blocks = {t: [] for t in tl}
        for b in range(B):
            call_b0, call_c0, nidx = blk - cb0, col - cc0, 0
            for t in tl:
                s = int(S_tb[t, b])
                sec_blk0[t, b] = blk
                sec_col0[t, b] = col
                tile_blocks[t].extend(range(blk - cb0, blk - cb0 + s // 128))
                blk += s // 128
                col += s // 16
                nidx += s
            # SWDGE ring = 1024 descriptors; a gather takes num_idxs/16+1
            # descs (multi-packet mode). Window well below the ring size.
            for off in range(0, nidx, 8192):
                p = min(8192, nidx - off)
                calls.append((b, call_c0 + off // 16, p, call_b0 + off // 128,
                              p // 128))
        chunks.append(dict(col0=cc0, cols=col - cc0, blk0=cb0, nblk=blk - cb0,
                           calls=calls, tiles=tl, tile_blocks=tile_blocks))

    lay = dict(
        chunks=chunks, NGRP=blk, TOTCOL=col,
        CBmax=max(ci["nblk"] for ci in chunks),
        COLmax=max(ci["cols"] for ci in chunks),
    )

    # per-edge slot assignment
    order = np.argsort(key, kind="stable")
    ksort = key[order]
    gstarts = np.zeros(C * T * B + 1, np.int64)
    np.cumsum(counts.reshape(-1), out=gstarts[1:])
    rank = np.arange(E, dtype=np.int64) - gstarts[ksort]
    et, eb = d_tile[order], s_bank[order]
    edge = dict(
        core=d_core[order],
        gblk=sec_blk0[et, eb] + rank // 128,
        prow=rank % 128,
        gcol=sec_col0[et, eb] + rank // 16,
        irow=rank % 16,
        code=d_code[order],
        rel=s_rel[order],
    )
    return lay, edge


def _prepare(cfg, inputs):
    x = np.ascontiguousarray(np.asarray(inputs["x"], np.float32))
    src = np.asarray(inputs["src"]).astype(np.int64)
    dst = np.asarray(inputs["dst"]).astype(np.int64)
    N, C, T, NPC, NPAD = cfg.N, cfg.C, cfg.T, cfg.NPC, cfg.NPAD

    deg = np.bincount(dst, minlength=N).astype(np.float32)
    norm = (np.clip(deg, 1.0, None) ** -0.5).astype(np.float32)
    nm2v = (norm * norm).astype(np.float32)

    lay, e = _layout(cfg, src, dst)

    codes = np.full((C, 128, lay["NGRP"]), 255, np.float32)
    codes[e["core"], e["prow"], e["gblk"]] = e["code"]
    codes_bf = codes.astype(NBF)

    idx16 = np.zeros((C, 16, lay["TOTCOL"]), np.int16)
    idx16[e["core"], e["irow"], e["gcol"]] = e["rel"].astype(np.int16)
    idx16 = np.ascontiguousarray(np.tile(idx16, (1, 8, 1)))

    def per_node_tiles(vec, pad, dt):
        a = np.full((C, NPAD), pad, np.float32)
        a[:, :NPC] = vec.reshape(C, NPC)
        return np.ascontiguousarray(a.reshape(C, T, 128).transpose(0, 2, 1)).astype(dt)

    nm_t = per_node_tiles(norm, 1.0, np.float32)
    nm2_t = per_node_tiles(nm2v, 1.0, np.float32)
    mask_t = per_node_tiles(np.ones(N, np.float32), 0.0, NBF)

    xp = np.zeros((C, NPAD, D), np.float32)
    xp[:, :NPC] = x.reshape(C, NPC, D)

    iotab = np.ascontiguousarray(
        np.broadcast_to(np.arange(128, dtype=np.float32), (128, 128))).astype(NBF)
    identb = np.eye(128, dtype=np.float32).astype(NBF)

    com = dict(
        iotab=iotab, identb=identb,
        W1b=np.asarray(inputs["W1"], np.float32).astype(NBF),
        b1b=np.asarray(inputs["b1"], np.float32).reshape(1, D).astype(NBF),
        W2b=np.asarray(inputs["W2"], np.float32).astype(NBF),
        b2b=np.asarray(inputs["b2"], np.float32).reshape(1, D).astype(NBF),
        Wf1f=np.ascontiguousarray(np.asarray(inputs["Wf1"], np.float32) / float(N)),
        bf1f=np.asarray(inputs["bf1"], np.float32).reshape(1, D).copy(),
        Wf2f=np.asarray(inputs["Wf2"], np.float32).reshape(D, 1).copy(),
        bf2f=np.asarray(inputs["bf2"], np.float32).reshape(1, 1).copy(),
    )
    in_maps = []
    for c in range(C):
        m = dict(com)
        m.update(x=np.ascontiguousarray(xp[c]),
                 nm=np.ascontiguousarray(nm_t[c]),
                 nm2=np.ascontiguousarray(nm2_t[c]),
                 maskb=np.ascontiguousarray(mask_t[c]),
                 codes=np.ascontiguousarray(codes_bf[c]),
                 idx=idx16[c])
        in_maps.append(m)
    return in_maps, lay


# --------------------------------------------------------------------------
# Kernel builder
# --------------------------------------------------------------------------

def _bcast_mid(ap_2d, n):
    """[P, W] AP -> [P, n, W] AP with a stride-0 middle dim."""
    return bass.AP(ap_2d.tensor, ap_2d.offset,
                   [ap_2d.ap[0], [0, n], ap_2d.ap[-1]])


def _bcast_inner(ap_2d, n):
    """[P, W] AP -> [P, W, n] AP with a stride-0 inner dim."""
    return bass.AP(ap_2d.tensor, ap_2d.offset,
                   [ap_2d.ap[0], ap_2d.ap[-1], [0, n]])


def _build(cfg, lay):
    C, T = cfg.C, cfg.T
    NPAD, NTOT, BROWS = cfg.NPAD, cfg.NTOT, cfg.BROWS
    RG = [list(range(C))]
    CBmax, COLmax = lay["CBmax"], lay["COLmax"]

    nc = bacc.Bacc("TRN2", target_bir_lowering=False, debug=False,
                   enable_asserts=False, num_devices=C,
                   num_swdge_queues=4)

    x_d = nc.dram_tensor("x", [NPAD, D], F32, kind="ExternalInput")
    nm_d = nc.dram_tensor("nm", [128, T], F32, kind="ExternalInput")
    nm2_d = nc.dram_tensor("nm2", [128, T], F32, kind="ExternalInput")
    mask_d = nc.dram_tensor("maskb", [128, T], BF16, kind="ExternalInput")
    codes_d = nc.dram_tensor("codes", [128, lay["NGRP"]], BF16, kind="ExternalInput")
    idx_d = nc.dram_tensor("idx", [128, lay["TOTCOL"]], I16, kind="ExternalInput")
    iota_d = nc.dram_tensor("iotab", [128, 128], BF16, kind="ExternalInput")
    ident_d = nc.dram_tensor("identb", [128, 128], BF16, kind="ExternalInput")
    W1_d = nc.dram_tensor("W1b", [D, D], BF16, kind="ExternalInput")
    b1_d = nc.dram_tensor("b1b", [1, D], BF16, kind="ExternalInput")
    W2_d = nc.dram_tensor("W2b", [D, D], BF16, kind="ExternalInput")
    b2_d = nc.dram_tensor("b2b", [1, D], BF16, kind="ExternalInput")
    Wf1_d = nc.dram_tensor("Wf1f", [D, D], F32, kind="ExternalInput")
    bf1_d = nc.dram_tensor("bf1f", [1, D], F32, kind="ExternalInput")
    Wf2_d = nc.dram_tensor("Wf2f", [D, 1], F32, kind="ExternalInput")
    bf2_d = nc.dram_tensor("bf2f", [1, 1], F32, kind="ExternalInput")
    out_d = nc.dram_tensor("out", [1, 1], F32, kind="ExternalOutput")

    with tile.TileContext(nc) as tc:
        with (
            tc.tile_pool(name="const", bufs=1) as const,
            tc.tile_pool(name="dloc", bufs=2, space="DRAM") as dloc,
            tc.tile_pool(name="dsh", bufs=2, space="DRAM") as dsh,
            tc.tile_pool(name="xp", bufs=3) as xp,
            tc.tile_pool(name="evp", bufs=3) as evp,
            tc.tile_pool(name="gp", bufs=2) as gp,
            tc.tile_pool(name="sp", bufs=2) as sp,
            tc.tile_pool(name="ipool", bufs=2) as ipool,
            tc.tile_pool(name="ph", bufs=3, space="PSUM") as ph,
            tc.tile_pool(name="px", bufs=2, space="PSUM") as px,
            tc.tile_pool(name="pp", bufs=1, space="PSUM") as pp,
        ):
            # ---- resident constants
            nm_t = const.tile([128, T], F32)
            nc.sync.dma_start(out=nm_t[:], in_=nm_d[:, :])
            nm2_t = const.tile([128, T], F32)
            nc.sync.dma_start(out=nm2_t[:], in_=nm2_d[:, :])
            mask_t = const.tile([128, T], BF16)
            nc.sync.dma_start(out=mask_t[:], in_=mask_d[:, :])
            codes_t = const.tile([128, lay["NGRP"]], BF16)
            nc.sync.dma_start(out=codes_t[:], in_=codes_d[:, :])
            iota_t = const.tile([128, 128], BF16)
            nc.sync.dma_start(out=iota_t[:], in_=iota_d[:, :])
            ident_t = const.tile([128, 128], BF16)
            nc.sync.dma_start(out=ident_t[:], in_=ident_d[:, :])
            w1_t = const.tile([D, D], BF16)
            nc.sync.dma_start(out=w1_t[:], in_=W1_d[:, :])
            b1_t = const.tile([1, D], BF16)
            nc.sync.dma_start(out=b1_t[:1, :], in_=b1_d[:, :])
            w2_t = const.tile([D, D], BF16)
            nc.sync.dma_start(out=w2_t[:], in_=W2_d[:, :])
            b2_t = const.tile([1, D], BF16)
            nc.sync.dma_start(out=b2_t[:1, :], in_=b2_d[:, :])
            wf1_t = const.tile([D, D], F32)
            nc.sync.dma_start(out=wf1_t[:], in_=Wf1_d[:, :])
            bf1_t = const.tile([1, D], F32)
            nc.sync.dma_start(out=bf1_t[:1, :], in_=bf1_d[:, :])
            wf2_t = const.tile([D, 1], F32)
            nc.sync.dma_start(out=wf2_t[:], in_=Wf2_d[:, :])
            bf2_t = const.tile([1, 1], F32)
            nc.sync.dma_start(out=bf2_t[:1, :1], in_=bf2_d[:, :])
            ones_b = const.tile([1, D], BF16)
            nc.vector.memset(ones_b[:1, :], 1.0)
            one11 = const.tile([1, 1], F32)
            nc.vector.memset(one11[:1, :1], 1.0)

            pool_ps = pp.tile([128, 1], F32)

            # ---- prologue: g0 = norm * x
            hloc = dloc.tile([NPAD, D], BF16, name="hloc")
            for t in range(T):
                xt = xp.tile([128, D], F32, name="xt")
                nc.sync.dma_start(out=xt[:], in_=x_d[t * 128:(t + 1) * 128, :])
                ev = evp.tile([128, D], BF16, name="ev")
                nc.scalar.activation(ev[:], xt[:], AF.Copy, scale=nm_t[:, t:t + 1])
                nc.sync.dma_start(out=hloc[t * 128:(t + 1) * 128, :], in_=ev[:])
            hfull = dsh.tile([NTOT, D], BF16, addr_space="Shared", name="hfull")
            nc.gpsimd.collective_compute(
                "AllGather", ALU.bypass, replica_groups=RG,
                ins=[hloc.opt()], outs=[hfull.opt()])

            # ---- 6 propagation hops
            qn = 0
            for hop in range(1, 7):
                boundary = hop in (3, 6)
                wt, bt = (w1_t, b1_t) if hop == 3 else (w2_t, b2_t)
                hl_new = None
                if hop < 6:
                    hl_new = dloc.tile([NPAD, D], BF16, name="hloc")
                for info in lay["chunks"]:
                    idxt = ipool.tile([128, COLmax], I16, name="idxt")
                    nc.sync.dma_start(
                        out=idxt[:, :info["cols"]],
                        in_=idx_d[:, info["col0"]:info["col0"] + info["cols"]])
                    G = gp.tile([128, CBmax, D], BF16, name="G")
                    for (b, ccol0, nidx, cblk0, nblk) in info["calls"]:
                        nc.gpsimd.dma_gather(
                            out_ap=G[:, cblk0:cblk0 + nblk, :],
                            in_ap=hfull[b * BROWS:(b + 1) * BROWS, :],
                            idxs_ap=idxt[:, ccol0:ccol0 + nidx // 16],
                            num_idxs=nidx, num_idxs_reg=nidx, elem_size=D,
                            single_packet=False, queue_num=qn % 4)
                        qn += 1
                    S = sp.tile([128, CBmax * D], BF16, name="S")
                    nb = info["nblk"]
                    s3 = S[:, :nb * D].rearrange("p (g d) -> p g d", d=D)
                    nc.vector.tensor_tensor(
                        s3, _bcast_mid(iota_t[:, :], nb),
                        _bcast_inner(codes_t[:, info["blk0"]:info["blk0"] + nb], D),
                        ALU.is_equal)
                    for t in info["tiles"]:
                        blks = info["tile_blocks"][t]
                        ps = ph.tile([128, D], F32, name="ps")
                        for j, bl in enumerate(blks):
                            nc.tensor.matmul(
                                ps[:], S[:, bl * D:(bl + 1) * D], G[:, bl, :],
                                start=(j == 0), stop=(j == len(blks) - 1))
                        if not boundary:
                            evh = evp.tile([128, D], BF16, name="evh")
                            nc.scalar.activation(evh[:], ps[:], AF.Copy,
                                                 scale=nm2_t[:, t:t + 1])
                            nc.sync.dma_start(
                                out=hl_new[t * 128:(t + 1) * 128, :], in_=evh[:])
                        else:
                            hv = evp.tile([128, D], BF16, name="hv")
                            nc.scalar.activation(hv[:], ps[:], AF.Copy,
                                                 scale=nm_t[:, t:t + 1])
                            tp = px.tile([128, D], BF16, name="tp")
                            nc.tensor.transpose(tp[:], hv[:], ident_t[:])
                            hT = evp.tile([128, D], BF16, name="hT")
                            nc.vector.tensor_copy(hT[:], tp[:])
                            wp = px.tile([128, D], F32, name="wp")
                            nc.tensor.matmul(wp[:], hT[:], wt[:],
                                             start=True, stop=False)
                            nc.tensor.matmul(wp[:], ones_b[:1, :], bt[:1, :],
                                             start=False, stop=True)
                            ev2 = evp.tile([128, D], BF16, name="ev2")
                            if hop == 3:
                                nc.scalar.activation(ev2[:], wp[:], AF.Relu,
                                                     scale=nm_t[:, t:t + 1])
                                nc.sync.dma_start(
                                    out=hl_new[t * 128:(t + 1) * 128, :], in_=ev2[:])
                            else:
                                nc.scalar.activation(ev2[:], wp[:], AF.Relu)
                                nc.tensor.matmul(
                                    pool_ps[:], ev2[:], mask_t[:, t:t + 1],
                                    start=(t == 0), stop=(t == T - 1),
                                    skip_group_check=True)
                if hop < 6:
                    hfull = dsh.tile([NTOT, D], BF16, addr_space="Shared",
                                     name="hfull")
                    nc.gpsimd.collective_compute(
                        "AllGather", ALU.bypass, replica_groups=RG,
                        ins=[hl_new.opt()], outs=[hfull.opt()])

            # ---- head: AllReduce pooled sums, 2-layer MLP, sigmoid
            hgs = evp.tile([128, 1], F32, name="hgs")
            nc.vector.tensor_copy(hgs[:], pool_ps[:])
            arin = dloc.tile([128, 1], F32, name="arin")
            nc.sync.dma_start(out=arin[:, :], in_=hgs[:])
            arout = dsh.tile([128, 1], F32, addr_space="Shared", name="arout")
            nc.gpsimd.collective_compute(
                "AllReduce", ALU.add, replica_groups=RG,
                ins=[arin.opt()], outs=[arout.opt()])
            hgT = evp.tile([128, 1], F32, name="hgT")
            nc.sync.dma_start(out=hgT[:, :], in_=arout[:, :])

            py1 = px.tile([1, D], F32, name="py1", tag="wp")
            nc.tensor.matmul(py1[:1, :], hgT[:], wf1_t[:],
                             start=True, stop=False, skip_group_check=True)
            nc.tensor.matmul(py1[:1, :], one11[:1, :1], bf1_t[:1, :],
                             start=False, stop=True, skip_group_check=True)
            y1 = evp.tile([1, D], F32, name="y1")
            nc.scalar.activation(y1[:1, :], py1[:1, :], AF.Relu)
            py1T = px.tile([128, 1], F32, name="py1T", tag="wp")
            nc.tensor.transpose(py1T[:, :1], y1[:1, :], one11[:1, :1])
            y1T = evp.tile([128, 1], F32, name="y1T")
            nc.vector.tensor_copy(y1T[:], py1T[:, :1])
            py2 = px.tile([1, 1], F32, name="py2", tag="wp")
            nc.tensor.matmul(py2[:1, :1], wf2_t[:], y1T[:],
                             start=True, stop=False, skip_group_check=True)
            nc.tensor.matmul(py2[:1, :1], one11[:1, :1], bf2_t[:1, :1],
                             start=False, stop=True, skip_group_check=True)
            r2 = evp.tile([1, 1], F32, name="r2")
            nc.scalar.activation(r2[:1, :1], py2[:1, :1], AF.Relu)
            osb = evp.tile([1, 1], F32, name="osb")
            nc.scalar.activation(osb[:1, :1], r2[:1, :1], AF.Sigmoid)
            nc.sync.dma_start(out=out_d[:, :], in_=osb[:1, :1])

    nc.compile()
    return nc


# --------------------------------------------------------------------------
# Runner
# --------------------------------------------------------------------------

_NC_CACHE = {}


def _ensure_ntff_hook():
    """bass_utils imports antenv.axon_hooks for trace=True under axon; the
    image's antenv lacks it. Provide it + register the ctypes NTFF hook."""
    import types
    try:
        from antenv import axon_hooks  # noqa: F401
        return
    except ImportError:
        pass
    import antenv
    mod = types.ModuleType("antenv.axon_hooks")
    _h = {"h": None}
    mod.set_axon_ntff_profile_hook = lambda h: _h.__setitem__("h", h)
    mod.get_axon_ntff_profile_hook = lambda: _h["h"]
    sys.modules["antenv.axon_hooks"] = mod
    antenv.axon_hooks = mod
    try:
        if "/root/.axon_site" not in sys.path:
            sys.path.insert(0, "/root/.axon_site")
        from trn_agent_boot.trn_boot import _ntff_profile_via_ctypes
        h = _ntff_profile_via_ctypes("/opt/axon/libaxon_pjrt.so")
        if h is not None:
            mod.set_axon_ntff_profile_hook(h)
    except Exception as e:  # pragma: no cover
        print("ntff hook setup failed:", e)


def _graph_key(cfg, src, dst):
    h = hashlib.sha1()
    h.update(np.asarray(src).tobytes())
    h.update(np.asarray(dst).tobytes())
    return (cfg.key(), h.hexdigest())


def run(inputs, cfg=None, trace=False, **kw):
    cfg = cfg or Cfg()
    if trace:
        _ensure_ntff_hook()
    in_maps, lay = _prepare(cfg, inputs)
    gk = _graph_key(cfg, inputs["src"], inputs["dst"])
    nc = _NC_CACHE.get(gk)
    if nc is None:
        nc = _build(cfg, lay)
        _NC_CACHE[gk] = nc
    res = bass_utils.run_bass_kernel_spmd(
        nc, in_maps, core_ids=list(range(cfg.C)), trace=trace, **kw)
    out = np.asarray(res.results[0]["out"], np.float32).reshape(1, 1)
    return out, res


def kernel(**inputs):
    out, _ = run(inputs)
    return out


# revision 12
# speedup vs baseline: 1.1512x; 1.0096x over previous
"""Trainium2 Bass kernel: BinaryGraphClassifier (2x SGConv(k=3) + MLP head).

Strategy (8 NeuronCores, SPMD):
  - Nodes partitioned contiguously across cores (12500/core, padded to 12544).
  - Per SpMM hop: AllGather full node table H (bf16) -> every core gathers its
    edges' source rows with dma_gather (edges pre-sorted by dst tile / src bank
    on host), segment-sums them into PSUM via one-hot matmuls on the PE, and
    rescales by the symmetric degree norm folded into per-partition ACT scales.
  - Boundaries (after hop 3 and 6): PE transpose + W matmul + bias + ReLU.
  - Mean pooling: mask-weighted column-sum matmul -> AllReduce -> tiny head.
"""

import os
import sys
import hashlib

import numpy as np
import ml_dtypes

for _p in ("/opt/trn_rl_repo", "/root/.axon_site/_ro/trn_rl_repo"):
    if os.path.isdir(_p) and _p not in sys.path:
        sys.path.insert(0, _p)

import concourse.bass as bass  # noqa: E402
import concourse.mybir as mybir  # noqa: E402
from concourse import bacc  # noqa: E402
from concourse import tile  # noqa: E402
from concourse import bass_utils  # noqa: E402

BF16 = mybir.dt.bfloat16
F32 = mybir.dt.float32
I16 = mybir.dt.int16
AF = mybir.ActivationFunctionType
ALU = mybir.AluOpType
NBF = ml_dtypes.bfloat16
D = 128


class Cfg:
    def __init__(self, N=100000, E=3200000, C=8, chunk=3, banks=4):
        self.N, self.E, self.C = N, E, C
        assert N % C == 0
        self.NPC = N // C                       # owned nodes per core
        self.T = (self.NPC + 127) // 128        # dst tiles per core
        self.NPAD = self.T * 128                # padded nodes per core
        self.NTOT = self.NPAD * C               # rows of the gathered table
        self.BANKS = banks
        assert self.NTOT % banks == 0
        self.BROWS = self.NTOT // banks         # rows per gather bank
        assert self.BROWS <= 32768              # int16 index limit
        self.CHUNK = chunk                      # dst tiles per gather chunk

    def key(self):
        return (self.N, self.E, self.C, self.CHUNK, self.BANKS)


# --------------------------------------------------------------------------
# Host-side graph preprocessing (partitioning / CSR-style edge layout)
# --------------------------------------------------------------------------

def _layout(cfg, src, dst):
    """Compute the shared (SPMD-identical) static layout + per-edge slots."""
    C, T, B, NPC, NPAD = cfg.C, cfg.T, cfg.BANKS, cfg.NPC, cfg.NPAD
    E = len(src)

    d_core = dst // NPC
    d_loc = dst - d_core * NPC
    d_tile = d_loc >> 7
    d_code = d_loc & 127
    s_core = src // NPC
    s_pad = s_core * NPAD + (src - s_core * NPC)
    s_bank = s_pad // cfg.BROWS
    s_rel = s_pad - s_bank * cfg.BROWS

    key = ((d_core * T + d_tile) * B + s_bank).astype(np.int64)
    counts = np.bincount(key, minlength=C * T * B).reshape(C, T, B)
    # Per-(tile,bank) section size, shared across cores: multiple of 128 so
    # every gather call starts on a 128-slot (one matmul group) boundary.
    S_tb = np.maximum(((counts.max(axis=0) + 127) // 128) * 128, 128)

    sec_blk0 = np.zeros((T, B), np.int64)
    sec_col0 = np.zeros((T, B), np.int64)
    chunks = []
    blk = col = 0
    for c0 in range(0, T, cfg.CHUNK):
        tl = list(range(c0, min(c0 + cfg.CHUNK, T)))
        cb0, cc0 = blk, col
        calls = []
        tile_blocks = {t: [] for t in tl}
        for b in range(B):
            call_b0, call_c0, nidx = blk - cb0, col - cc0, 0
            for t in tl:
                s = int(S_tb[t, b])
                sec_blk0[t, b] = blk
                sec_col0[t, b] = col
                tile_blocks[t].extend(range(blk - cb0, blk - cb0 + s // 128))
                blk += s // 128
                col += s // 16
                nidx += s
            # SWDGE ring = 1024 descriptors; a gather takes num_idxs/16+1
            # descs (multi-packet mode). Window well below the ring size.
            for off in range(0, nidx, 8192):
                p = min(8192, nidx - off)
                calls.append((b, call_c0 + off // 16, p, call_b0 + off // 128,
                              p // 128))
        chunks.append(dict(col0=cc0, cols=col - cc0, blk0=cb0, nblk=blk - cb0,
                           calls=calls, tiles=tl, tile_blocks=tile_blocks))

    lay = dict(
        chunks=chunks, NGRP=blk, TOTCOL=col,
        CBmax=max(ci["nblk"] for ci in chunks),
        COLmax=max(ci["cols"] for ci in chunks),
    )

    # per-edge slot assignment
    order = np.argsort(key, kind="stable")
    ksort = key[order]
    gstarts = np.zeros(C * T * B + 1, np.int64)
    np.cumsum(counts.reshape(-1), out=gstarts[1:])
    rank = np.arange(E, dtype=np.int64) - gstarts[ksort]
    et, eb = d_tile[order], s_bank[order]
    edge = dict(
        core=d_core[order],
        gblk=sec_blk0[et, eb] + rank // 128,
        prow=rank % 128,
        gcol=sec_col0[et, eb] + rank // 16,
        irow=rank % 16,
        code=d_code[order],
        rel=s_rel[order],
    )
    return lay, edge


def _prepare(cfg, inputs):
    x = np.ascontiguousarray(np.asarray(inputs["x"], np.float32))
    src = np.asarray(inputs["src"]).astype(np.int64)
    dst = np.asarray(inputs["dst"]).astype(np.int64)
    N, C, T, NPC, NPAD = cfg.N, cfg.C, cfg.T, cfg.NPC, cfg.NPAD

    deg = np.bincount(dst, minlength=N).astype(np.float32)
    norm = (np.clip(deg, 1.0, None) ** -0.5).astype(np.float32)
    nm2v = (norm * norm).astype(np.float32)

    lay, e = _layout(cfg, src, dst)

    codes = np.full((C, 128, lay["NGRP"]), 255, np.float32)
    codes[e["core"], e["prow"], e["gblk"]] = e["code"]
    codes_bf = codes.astype(NBF)

    idx16 = np.zeros((C, 16, lay["TOTCOL"]), np.int16)
    idx16[e["core"], e["irow"], e["gcol"]] = e["rel"].astype(np.int16)
    idx16 = np.ascontiguousarray(np.tile(idx16, (1, 8, 1)))

    def per_node_tiles(vec, pad, dt):
        a = np.full((C, NPAD), pad, np.float32)
        a[:, :NPC] = vec.reshape(C, NPC)
        return np.ascontiguousarray(a.reshape(C, T, 128).transpose(0, 2, 1)).astype(dt)

    nm_t = per_node_tiles(norm, 1.0, np.float32)
    nm2_t = per_node_tiles(nm2v, 1.0, np.float32)
    mask_t = per_node_tiles(np.ones(N, np.float32), 0.0, NBF)

    xp = np.zeros((C, NPAD, D), np.float32)
    xp[:, :NPC] = x.reshape(C, NPC, D)

    iotab = np.ascontiguousarray(
        np.broadcast_to(np.arange(128, dtype=np.float32), (128, 128))).astype(NBF)
    identb = np.eye(128, dtype=np.float32).astype(NBF)

    com = dict(
        iotab=iotab, identb=identb,
        W1b=np.asarray(inputs["W1"], np.float32).astype(NBF),
        b1b=np.asarray(inputs["b1"], np.float32).reshape(1, D).astype(NBF),
        W2b=np.asarray(inputs["W2"], np.float32).astype(NBF),
        b2b=np.asarray(inputs["b2"], np.float32).reshape(1, D).astype(NBF),
        Wf1f=np.ascontiguousarray(np.asarray(inputs["Wf1"], np.float32) / float(N)),
        bf1f=np.asarray(inputs["bf1"], np.float32).reshape(1, D).copy(),
        Wf2f=np.asarray(inputs["Wf2"], np.float32).reshape(D, 1).copy(),
        bf2f=np.asarray(inputs["bf2"], np.float32).reshape(1, 1).copy(),
    )
    in_maps = []
    for c in range(C):
        m = dict(com)
        m.update(x=np.ascontiguousarray(xp[c]),
                 nm=np.ascontiguousarray(nm_t[c]),
                 nm2=np.ascontiguousarray(nm2_t[c]),
                 maskb=np.ascontiguousarray(mask_t[c]),
                 codes=np.ascontiguousarray(codes_bf[c]),
                 idx=idx16[c])
        in_maps.append(m)
    return in_maps, lay


# --------------------------------------------------------------------------
# Kernel builder
# --------------------------------------------------------------------------

def _bcast_mid(ap_2d, n):
    """[P, W] AP -> [P, n, W] AP with a stride-0 middle dim."""
    return bass.AP(ap_2d.tensor, ap_2d.offset,
                   [ap_2d.ap[0], [0, n], ap_2d.ap[-1]])


def _bcast_inner(ap_2d, n):
    """[P, W] AP -> [P, W, n] AP with a stride-0 inner dim."""
    return bass.AP(ap_2d.tensor, ap_2d.offset,
                   [ap_2d.ap[0], ap_2d.ap[-1], [0, n]])


def _build(cfg, lay):
    C, T = cfg.C, cfg.T
    NPAD, NTOT, BROWS = cfg.NPAD, cfg.NTOT, cfg.BROWS
    RG = [list(range(C))]
    CBmax, COLmax = lay["CBmax"], lay["COLmax"]

    nc = bacc.Bacc("TRN2", target_bir_lowering=False, debug=False,
                   enable_asserts=False, num_devices=C,
                   num_swdge_queues=4, dynamic_dma_scratch_size=32768)

    x_d = nc.dram_tensor("x", [NPAD, D], F32, kind="ExternalInput")
    nm_d = nc.dram_tensor("nm", [128, T], F32, kind="ExternalInput")
    nm2_d = nc.dram_tensor("nm2", [128, T], F32, kind="ExternalInput")
    mask_d = nc.dram_tensor("maskb", [128, T], BF16, kind="ExternalInput")
    codes_d = nc.dram_tensor("codes", [128, lay["NGRP"]], BF16, kind="ExternalInput")
    idx_d = nc.dram_tensor("idx", [128, lay["TOTCOL"]], I16, kind="ExternalInput")
    iota_d = nc.dram_tensor("iotab", [128, 128], BF16, kind="ExternalInput")
    ident_d = nc.dram_tensor("identb", [128, 128], BF16, kind="ExternalInput")
    W1_d = nc.dram_tensor("W1b", [D, D], BF16, kind="ExternalInput")
    b1_d = nc.dram_tensor("b1b", [1, D], BF16, kind="ExternalInput")
    W2_d = nc.dram_tensor("W2b", [D, D], BF16, kind="ExternalInput")
    b2_d = nc.dram_tensor("b2b", [1, D], BF16, kind="ExternalInput")
    Wf1_d = nc.dram_tensor("Wf1f", [D, D], F32, kind="ExternalInput")
    bf1_d = nc.dram_tensor("bf1f", [1, D], F32, kind="ExternalInput")
    Wf2_d = nc.dram_tensor("Wf2f", [D, 1], F32, kind="ExternalInput")
    bf2_d = nc.dram_tensor("bf2f", [1, 1], F32, kind="ExternalInput")
    out_d = nc.dram_tensor("out", [1, 1], F32, kind="ExternalOutput")

    with tile.TileContext(nc) as tc:
        with (
            tc.tile_pool(name="const", bufs=1) as const,
            tc.tile_pool(name="dloc", bufs=2, space="DRAM") as dloc,
            tc.tile_pool(name="dsh", bufs=2, space="DRAM") as dsh,
            tc.tile_pool(name="xp", bufs=3) as xp,
            tc.tile_pool(name="evp", bufs=3) as evp,
            tc.tile_pool(name="gp", bufs=2) as gp,
            tc.tile_pool(name="sp", bufs=2) as sp,
            tc.tile_pool(name="ipool", bufs=2) as ipool,
            tc.tile_pool(name="ph", bufs=3, space="PSUM") as ph,
            tc.tile_pool(name="px", bufs=2, space="PSUM") as px,
            tc.tile_pool(name="pp", bufs=1, space="PSUM") as pp,
        ):
            # ---- resident constants
            nm_t = const.tile([128, T], F32)
            nc.sync.dma_start(out=nm_t[:], in_=nm_d[:, :])
            nm2_t = const.tile([128, T], F32)
            nc.sync.dma_start(out=nm2_t[:], in_=nm2_d[:, :])
            mask_t = const.tile([128, T], BF16)
            nc.sync.dma_start(out=mask_t[:], in_=mask_d[:, :])
            codes_t = const.tile([128, lay["NGRP"]], BF16)
            nc.sync.dma_start(out=codes_t[:], in_=codes_d[:, :])
            iota_t = const.tile([128, 128], BF16)
            nc.sync.dma_start(out=iota_t[:], in_=iota_d[:, :])
            ident_t = const.tile([128, 128], BF16)
            nc.sync.dma_start(out=ident_t[:], in_=ident_d[:, :])
            w1_t = const.tile([D, D], BF16)
            nc.sync.dma_start(out=w1_t[:], in_=W1_d[:, :])
            b1_t = const.tile([1, D], BF16)
            nc.sync.dma_start(out=b1_t[:1, :], in_=b1_d[:, :])
            w2_t = const.tile([D, D], BF16)
            nc.sync.dma_start(out=w2_t[:], in_=W2_d[:, :])
            b2_t = const.tile([1, D], BF16)
            nc.sync.dma_start(out=b2_t[:1, :], in_=b2_d[:, :])
            wf1_t = const.tile([D, D], F32)
            nc.sync.dma_start(out=wf1_t[:], in_=Wf1_d[:, :])
            bf1_t = const.tile([1, D], F32)
            nc.sync.dma_start(out=bf1_t[:1, :], in_=bf1_d[:, :])
            wf2_t = const.tile([D, 1], F32)
            nc.sync.dma_start(out=wf2_t[:], in_=Wf2_d[:, :])
            bf2_t = const.tile([1, 1], F32)
            nc.sync.dma_start(out=bf2_t[:1, :1], in_=bf2_d[:, :])
            ones_b = const.tile([1, D], BF16)
            nc.vector.memset(ones_b[:1, :], 1.0)
            one11 = const.tile([1, 1], F32)
            nc.vector.memset(one11[:1, :1], 1.0)

            pool_ps = pp.tile([128, 1], F32)

            # ---- prologue: g0 = norm * x
            hloc = dloc.tile([NPAD, D], BF16, name="hloc")
            for t in range(T):
                xt = xp.tile([128, D], F32, name="xt")
                nc.sync.dma_start(out=xt[:], in_=x_d[t * 128:(t + 1) * 128, :])
                ev = evp.tile([128, D], BF16, name="ev")
                nc.scalar.activation(ev[:], xt[:], AF.Copy, scale=nm_t[:, t:t + 1])
                nc.sync.dma_start(out=hloc[t * 128:(t + 1) * 128, :], in_=ev[:])
            hfull = dsh.tile([NTOT, D], BF16, addr_space="Shared", name="hfull")
            nc.gpsimd.collective_compute(
                "AllGather", ALU.bypass, replica_groups=RG,
                ins=[hloc.opt()], outs=[hfull.opt()])

            # ---- 6 propagation hops
            qn = 0
            for hop in range(1, 7):
                boundary = hop in (3, 6)
                wt, bt = (w1_t, b1_t) if hop == 3 else (w2_t, b2_t)
                hl_new = None
                if hop < 6:
                    hl_new = dloc.tile([NPAD, D], BF16, name="hloc")
                for info in lay["chunks"]:
                    idxt = ipool.tile([128, COLmax], I16, name="idxt")
                    nc.sync.dma_start(
                        out=idxt[:, :info["cols"]],
                        in_=idx_d[:, info["col0"]:info["col0"] + info["cols"]])
                    G = gp.tile([128, CBmax, D], BF16, name="G")
                    for (b, ccol0, nidx, cblk0, nblk) in info["calls"]:
                        nc.gpsimd.dma_gather(
                            out_ap=G[:, cblk0:cblk0 + nblk, :],
                            in_ap=hfull[b * BROWS:(b + 1) * BROWS, :],
                            idxs_ap=idxt[:, ccol0:ccol0 + nidx // 16],
                            num_idxs=nidx, num_idxs_reg=nidx, elem_size=D,
                            single_packet=False, queue_num=qn % 4)
                        qn += 1
                    S = sp.tile([128, CBmax * D], BF16, name="S")
                    nb = info["nblk"]
                    s3 = S[:, :nb * D].rearrange("p (g d) -> p g d", d=D)
                    nc.vector.tensor_tensor(
                        s3, _bcast_mid(iota_t[:, :], nb),
                        _bcast_inner(codes_t[:, info["blk0"]:info["blk0"] + nb], D),
                        ALU.is_equal)
                    for t in info["tiles"]:
                        blks = info["tile_blocks"][t]
                        ps = ph.tile([128, D], F32, name="ps")
                        for j, bl in enumerate(blks):
                            nc.tensor.matmul(
                                ps[:], S[:, bl * D:(bl + 1) * D], G[:, bl, :],
                                start=(j == 0), stop=(j == len(blks) - 1))
                        if not boundary:
                            evh = evp.tile([128, D], BF16, name="evh")
                            nc.scalar.activation(evh[:], ps[:], AF.Copy,
                                                 scale=nm2_t[:, t:t + 1])
                            nc.sync.dma_start(
                                out=hl_new[t * 128:(t + 1) * 128, :], in_=evh[:])
                        else:
                            hv = evp.tile([128, D], BF16, name="hv")
                            nc.scalar.activation(hv[:], ps[:], AF.Copy,
                                                 scale=nm_t[:, t:t + 1])
                            tp = px.tile([128, D], BF16, name="tp")
                            nc.tensor.transpose(tp[:], hv[:], ident_t[:])
                            hT = evp.tile([128, D], BF16, name="hT")
                            nc.vector.tensor_copy(hT[:], tp[:])
                            wp = px.tile([128, D], F32, name="wp")
                            nc.tensor.matmul(wp[:], hT[:], wt[:],
                                             start=True, stop=False)
                            nc.tensor.matmul(wp[:], ones_b[:1, :], bt[:1, :],
                                             start=False, stop=True)
                            ev2 = evp.tile([128, D], BF16, name="ev2")
                            if hop == 3:
                                nc.scalar.activation(ev2[:], wp[:], AF.Relu,
                                                     scale=nm_t[:, t:t + 1])
                                nc.sync.dma_start(
                                    out=hl_new[t * 128:(t + 1) * 128, :], in_=ev2[:])
                            else:
                                nc.scalar.activation(ev2[:], wp[:], AF.Relu)
                                nc.tensor.matmul(
                                    pool_ps[:], ev2[:], mask_t[:, t:t + 1],
                                    start=(t == 0), stop=(t == T - 1),
                                    skip_group_check=True)
                if hop < 6:
                    hfull = dsh.tile([NTOT, D], BF16, addr_space="Shared",
                                     name="hfull")
                    nc.gpsimd.collective_compute(
                        "AllGather", ALU.bypass, replica_groups=RG,
                        ins=[hl_new.opt()], outs=[hfull.opt()])

            # ---- head: AllReduce pooled sums, 2-layer MLP, sigmoid
            hgs = evp.tile([128, 1], F32, name="hgs")
            nc.vector.tensor_copy(hgs[:], pool_ps[:])
            arin = dloc.tile([128, 1], F32, name="arin")
            nc.sync.dma_start(out=arin[:, :], in_=hgs[:])
            arout = dsh.tile([128, 1], F32, addr_space="Shared", name="arout")
            nc.gpsimd.collective_compute(
                "AllReduce", ALU.add, replica_groups=RG,
                ins=[arin.opt()], outs=[arout.opt()])
            hgT = evp.tile([128, 1], F32, name="hgT")
            nc.sync.dma_start(out=hgT[:, :], in_=arout[:, :])

            py1 = px.tile([1, D], F32, name="py1", tag="wp")
            nc.tensor.matmul(py1[:1, :], hgT[:], wf1_t[:],
                             start=True, stop=False, skip_group_check=True)
            nc.tensor.matmul(py1[:1, :], one11[:1, :1], bf1_t[:1, :],
                             start=False, stop=True, skip_group_check=True)
            y1 = evp.tile([1, D], F32, name="y1")
            nc.scalar.activation(y1[:1, :], py1[:1, :], AF.Relu)
            py1T = px.tile([128, 1], F32, name="py1T", tag="wp")
            nc.tensor.transpose(py1T[:, :1], y1[:1, :], one11[:1, :1])
            y1T = evp.tile([128, 1], F32, name="y1T")
            nc.vector.tensor_copy(y1T[:], py1T[:, :1])
            py2 = px.tile([1, 1], F32, name="py2", tag="wp")
            nc.tensor.matmul(py2[:1, :1], wf2_t[:], y1T[:],
                             start=True, stop=False, skip_group_check=True)
            nc.tensor.matmul(py2[:1, :1], one11[:1, :1], bf2_t[:1, :1],
                             start=False, stop=True, skip_group_check=True)
            r2 = evp.tile([1, 1], F32, name="r2")
            nc.scalar.activation(r2[:1, :1], py2[:1, :1], AF.Relu)
            osb = evp.tile([1, 1], F32, name="osb")
            nc.scalar.activation(osb[:1, :1], r2[:1, :1], AF.Sigmoid)
            nc.sync.dma_start(out=out_d[:, :], in_=osb[:1, :1])

    nc.compile()
    return nc


# --------------------------------------------------------------------------
# Runner
# --------------------------------------------------------------------------

_NC_CACHE = {}


def _ensure_ntff_hook():
    """bass_utils imports antenv.axon_hooks for trace=True under axon; the
    image's antenv lacks it. Provide it + register the ctypes NTFF hook."""
    import types
    try:
        from antenv import axon_hooks  # noqa: F401
        return
    except ImportError:
        pass
    import antenv
    mod = types.ModuleType("antenv.axon_hooks")
    _h = {"h": None}
    mod.set_axon_ntff_profile_hook = lambda h: _h.__setitem__("h", h)
    mod.get_axon_ntff_profile_hook = lambda: _h["h"]
    sys.modules["antenv.axon_hooks"] = mod
    antenv.axon_hooks = mod
    try:
        if "/root/.axon_site" not in sys.path:
            sys.path.insert(0, "/root/.axon_site")
        from trn_agent_boot.trn_boot import _ntff_profile_via_ctypes
        h = _ntff_profile_via_ctypes("/opt/axon/libaxon_pjrt.so")
        if h is not None:
            mod.set_axon_ntff_profile_hook(h)
    except Exception as e:  # pragma: no cover
        print("ntff hook setup failed:", e)


def _graph_key(cfg, src, dst):
    h = hashlib.sha1()
    h.update(np.asarray(src).tobytes())
    h.update(np.asarray(dst).tobytes())
    return (cfg.key(), h.hexdigest())


def run(inputs, cfg=None, trace=False, **kw):
    cfg = cfg or Cfg()
    if trace:
        _ensure_ntff_hook()
    in_maps, lay = _prepare(cfg, inputs)
    gk = _graph_key(cfg, inputs["src"], inputs["dst"])
    nc = _NC_CACHE.get(gk)
    if nc is None:
        nc = _build(cfg, lay)
        _NC_CACHE[gk] = nc
    res = bass_utils.run_bass_kernel_spmd(
        nc, in_maps, core_ids=list(range(cfg.C)), trace=trace, **kw)
    out = np.asarray(res.results[0]["out"], np.float32).reshape(1, 1)
    return out, res


def kernel(**inputs):
    out, _ = run(inputs)
    return out
